# revision 1
# baseline (speedup 1.0000x reference)
"""Trainium2 Bass kernel: dual-attention transformer block (nn_CustomBlock).

Reference semantics (per batch element b):
    q/k/v = x_b @ sa_w{q,k,v} + sa_b{q,k,v}
    sa    = softmax(q k^T / sqrt(DB)) v @ sa_wo + sa_bo
    x_b1  = x_b + sa
    q     = x_a @ ca_wq + ca_bq ; k/v = x_b1 @ ca_w{k,v} + ca_b{k,v}
    out   = x_b1 + softmax(q k^T / sqrt(DA)) v @ ca_wo + ca_bo

Sharding: data-parallel over batch — 8 batch elements, one per NeuronCore,
weights replicated.  No collectives.

Device kernel works in bf16 for all matmul operands (fp32 PSUM accumulation,
fp32 residual stream).  Exact host-side bias folding:
  - k-bias shifts every score row by a constant -> softmax-invariant -> dropped.
  - v-bias passes through attention unchanged (softmax weights sum to 1), so
    bv @ wo + bo folds into a single per-feature vector added to the residual
    input (SA) / the final output (CA) on the host.
  - q-bias is applied on device (per-partition bias in the q^T layout).

Softmax skips the max-subtraction: scores = q.k/sqrt(D) with these operand
scales stays in [-3, 3]; exp() in fp32 is safe by a wide margin.
"""

import math
import os
from contextlib import ExitStack

import numpy as np
import ml_dtypes

import concourse.bass as bass
import concourse.mybir as mybir
import concourse.tile as tile
from concourse import bacc
from concourse.bass_utils import run_bass_kernel_spmd

P = 128
F32 = mybir.dt.float32
BF16 = mybir.dt.bfloat16
AF = mybir.ActivationFunctionType
ALU = mybir.AluOpType

B_FULL, N_FULL, DA_FULL, DB_FULL = 8, 2048, 768, 1024


def build_block(tc, outs, ins, n, da, db):
    """Emit the dual-attention block into TileContext `tc`.

    ins/outs: dicts of DRAM APs:
      ins:  xb_bf [n,db] bf16, xa_bf [n,da] bf16, xbpb [n,db] f32,
            sa_wq/sa_wk/sa_wv/sa_wo [db,db] bf16, ca_wq [da,db] bf16,
            ca_wk/ca_wv/ca_wo [db,db] bf16, bq_sa [P,db/P] f32, bq_ca [P,db/P] f32
      outs: out [n,db] f32
    """
    nc = tc.nc
    KB, KA, NI = db // P, da // P, n // P
    MC = min(1024, n)         # projection m-chunk (columns of x^T); 2 psum banks
    NMC = n // MC
    PC = min(512, MC)         # one psum bank within a projection chunk
    NPC = MC // PC
    JH = min(1024, n)         # scores psum span (2 banks)
    NJH = n // JH
    JC = min(512, JH)         # one psum bank
    NJC = JH // JC
    SB = min(512, n)          # attention superblock (i columns per AV batch)
    NSB = n // SB
    IPSB = SB // P            # i-blocks per superblock
    EC = min(512, db)         # out-proj free chunk
    NEC = db // EC

    sc_sa = 1.0 / math.sqrt(float(db))
    sc_ca = 1.0 / math.sqrt(float(da))

    ctx = ExitStack()
    with ctx:
        sp = ctx.enter_context(tc.tile_pool(name="sp", bufs=1))
        pp = ctx.enter_context(tc.tile_pool(name="pp", bufs=1, space="PSUM"))
        dp = ctx.enter_context(tc.tile_pool(name="dp", bufs=1, space="DRAM"))

        # DRAM scratch
        qt_sa_d = dp.tile([db, n], BF16, tag="qt_sa")
        qt_ca_d = dp.tile([db, n], BF16, tag="qt_ca")
        xb1_d = dp.tile([n, db], F32, tag="xb1")
        xb1b_d = dp.tile([n, db], BF16, tag="xb1b")

        # persistent SBUF
        kT = sp.tile([P, KB, n], BF16, tag="kT")        # k^T  [feat, seq]
        v_sb = sp.tile([P, NI, db], BF16, tag="v")      # v    [seq, feat]
        bqs = sp.tile([P, KB], F32, tag="bqs")
        bqc = sp.tile([P, KB], F32, tag="bqc")
        zb = sp.tile([P, 1], F32, tag="zb")
        nc.sync.dma_start(bqs[:], ins["bq_sa"][:])
        nc.sync.dma_start(bqc[:], ins["bq_ca"][:])
        nc.gpsimd.memset(zb[:], 0.0)

        def load_w(name, ktiles):
            # two half-loads: consumers of the first k-tiles start after 1MB,
            # not the whole matrix (Tile tracks subtile write regions)
            wt = sp.tile([P, ktiles, db], BF16, tag="w", bufs=2)
            src = ins[name].rearrange("(t p) e -> p t e", p=P)
            h = max(1, ktiles // 2)
            nc.sync.dma_start(wt[:, :h, :], src[:, :h, :])
            if h < ktiles:
                nc.sync.dma_start(wt[:, h:, :], src[:, h:, :])
            return wt

        def xpose_chunk(src_dram, ktiles, mcc):
            # x [mc-chunk, k] -> x^T chunk [p, kt, m] with k = kt*P + p
            # (tag shared with the attention wT superblock tiles: the phases
            # are sequential, and sharing keeps total SBUF under the cap)
            xT = sp.tile([P, ktiles, MC], BF16, tag="xcwt", bufs=2)
            nc.sync.dma_start_transpose(xT[:], src_dram[mcc * MC:(mcc + 1) * MC, :])
            return xT

        def proj_v(w_sb, src_dram, ktiles):
            # v[m, e] = sum_k x[m, k] w[k, e]  (natural layout, into v_sb).
            # One [P, db] psum spans all e-chunks: each LDWEIGHTS (the x-slice)
            # serves NEC matmuls instead of one.
            for mcc in range(NMC):
                xT = xpose_chunk(src_dram, ktiles, mcc)
                for q2 in range(MC // P):
                    mt = mcc * (MC // P) + q2
                    ps = pp.tile([P, db], F32, tag="ps_s", bufs=2)
                    for kt in range(ktiles):
                        for ecc in range(NEC):
                            nc.tensor.matmul(
                                ps[:, ecc * EC:(ecc + 1) * EC],
                                xT[:, kt, q2 * P:(q2 + 1) * P],
                                w_sb[:, kt, ecc * EC:(ecc + 1) * EC],
                                start=(kt == 0), stop=(kt == ktiles - 1),
                            )
                    nc.vector.tensor_copy(v_sb[:, mt, :], ps[:])

        def proj_T_block(w_sb, ktiles, xT, nt, mcc, sink):
            # out^T[f, m] = sum_k w[k, f] x^T[k, m] for f-tile nt, m-chunk mcc.
            # One [P, MC] psum spans NPC m-halves: each LDWEIGHTS (the w-slice)
            # serves NPC matmuls instead of one.
            ps = pp.tile([P, MC], F32, tag="ps_s", bufs=2)
            for kt in range(ktiles):
                for jc in range(NPC):
                    nc.tensor.matmul(
                        ps[:, jc * PC:(jc + 1) * PC],
                        w_sb[:, kt, nt * P:(nt + 1) * P],
                        xT[:, kt, jc * PC:(jc + 1) * PC],
                        start=(kt == 0), stop=(kt == ktiles - 1),
                    )
            sink(nt, mcc, ps)

        def q_sink(qt_d, bq_tile):
            def sink(nt, mcc, ps):
                qo = sp.tile([P, MC], BF16, tag="qv", bufs=2)
                nc.scalar.activation(qo[:], ps[:], AF.Identity, bias=bq_tile[:, nt:nt + 1])
                nc.sync.dma_start(qt_d[nt * P:(nt + 1) * P, mcc * MC:(mcc + 1) * MC], qo[:])
            return sink

        def k_sink(nt, mcc, ps):
            nc.vector.tensor_copy(kT[:, nt, mcc * MC:(mcc + 1) * MC], ps[:])

        def attention(qt_d, scale, wo_sb, resid_dram, writer):
            # Software-pipelined over superblocks: the scores/softmax/transpose
            # chain of superblock sbi is emitted BEFORE the AV/out-proj of
            # sbi-1, so the PE never stalls on the (ACT/DVE/DMA) softmax tail.
            def scores_phase(sbi, wt_t):
                for q3 in range(IPSB):
                    ib = sbi * IPSB + q3
                    qs_t = sp.tile([P, KB, P], BF16, tag="qs", bufs=2)
                    nc.sync.dma_start(
                        qs_t[:],
                        qt_d.rearrange("(t p) m -> p t m", p=P)[:, :, ib * P:(ib + 1) * P],
                    )
                    wb_t = sp.tile([P, n], BF16, tag="wb", bufs=2)
                    ss_t = sp.tile([P, NJH], F32, tag="ss", bufs=2)
                    for jh in range(NJH):
                        ps_s = pp.tile([P, JH], F32, tag="ps_s", bufs=2)
                        for kt in range(KB):
                            for jc in range(NJC):
                                nc.tensor.matmul(
                                    ps_s[:, jc * JC:(jc + 1) * JC],
                                    qs_t[:, kt, :],
                                    kT[:, kt, jh * JH + jc * JC:jh * JH + (jc + 1) * JC],
                                    start=(kt == 0), stop=(kt == KB - 1),
                                )
                        nc.scalar.activation(
                            wb_t[:, jh * JH:(jh + 1) * JH], ps_s[:], AF.Exp,
                            bias=zb[:], scale=scale,
                            accum_out=ss_t[:, jh:jh + 1],
                        )
                    rr = sp.tile([P, 1], F32, tag="rr", bufs=2)
                    if NJH > 1:
                        rs = sp.tile([P, 1], F32, tag="rs", bufs=2)
                        nc.vector.tensor_reduce(rs[:], ss_t[:], axis=mybir.AxisListType.X, op=ALU.add)
                        nc.vector.reciprocal(rr[:], rs[:])
                    else:
                        nc.vector.reciprocal(rr[:], ss_t[:])
                    nc.vector.tensor_scalar_mul(wb_t[:], wb_t[:], rr[:, 0:1])
                    # transpose the normalized weights: w[i, j] -> wT[j, i]
                    wtb = sp.tile([P, NI, P], BF16, tag="wtb", bufs=2)
                    nc.sync.dma_start_transpose(wtb[:], wb_t[:])
                    nc.vector.tensor_copy(wt_t[:, :, q3 * P:(q3 + 1) * P], wtb[:])

            def av_part(sbi, wt_t):
                # attn^T[d, i] = sum_j v[j, d] wT[j, i]
                at_t = sp.tile([P, KB, SB], BF16, tag="at", bufs=2)
                for dt in range(KB):
                    ps_a = pp.tile([P, SB], F32, tag="ps_a", bufs=2)
                    for jt in range(NI):
                        nc.tensor.matmul(
                            ps_a[:],
                            v_sb[:, jt, dt * P:(dt + 1) * P],
                            wt_t[:, jt, :],
                            start=(jt == 0), stop=(jt == NI - 1),
                        )
                    nc.vector.tensor_copy(at_t[:, dt, :], ps_a[:])
                return at_t

            def op_part(sbi, at_t):
                # out-proj + residual
                for q3 in range(IPSB):
                    ib = sbi * IPSB + q3
                    rx = sp.tile([P, db], F32, tag="rx", bufs=2)
                    nc.sync.dma_start(rx[:], resid_dram[ib * P:(ib + 1) * P, :])
                    ro = sp.tile([P, db], F32, tag="ro", bufs=2)
                    for ecc in range(NEC):
                        ps_o = pp.tile([P, EC], F32, tag="pj", bufs=2)
                        for dt in range(KB):
                            nc.tensor.matmul(
                                ps_o[:],
                                at_t[:, dt, q3 * P:(q3 + 1) * P],
                                wo_sb[:, dt, ecc * EC:(ecc + 1) * EC],
                                start=(dt == 0), stop=(dt == KB - 1),
                            )
                        nc.vector.tensor_tensor(
                            ro[:, ecc * EC:(ecc + 1) * EC], ps_o[:],
                            rx[:, ecc * EC:(ecc + 1) * EC], ALU.add,
                        )
                    writer(ib, ro)

            pend_av = None   # (sbi, wt_t) awaiting AV
            pend_op = None   # (sbi, at_t) awaiting out-proj
            for sbi in range(NSB):
                wt_t = sp.tile([P, NI, SB], BF16, tag="xcwt", bufs=2)
                scores_phase(sbi, wt_t)
                new_at = av_part(*pend_av) if pend_av is not None else None
                if pend_op is not None:
                    op_part(*pend_op)
                if new_at is not None:
                    pend_op = (pend_av[0], new_at)
                pend_av = (sbi, wt_t)
            at_t = av_part(*pend_av)
            if pend_op is not None:
                op_part(*pend_op)
            op_part(pend_av[0], at_t)

        def sa_writer(ib, ro):
            nc.sync.dma_start(xb1_d[ib * P:(ib + 1) * P, :], ro[:])
            rb = sp.tile([P, db], BF16, tag="rb", bufs=2)
            nc.scalar.activation(rb[:], ro[:], AF.Copy)
            nc.sync.dma_start(xb1b_d[ib * P:(ib + 1) * P, :], rb[:])

        def ca_writer(ib, ro):
            nc.sync.dma_start(outs["out"][ib * P:(ib + 1) * P, :], ro[:])

        # CA-q depends only on x_a — emit it first: smallest startup loads
        # (1.5MB weight + 1.5MB first transpose), and it decouples the
        # SA->CA boundary entirely.
        wq2 = load_w("ca_wq", KA)
        sink_q_ca = q_sink(qt_ca_d, bqc)
        for mcc in range(NMC):
            xTa = xpose_chunk(ins["xa_bf"], KA, mcc)
            for nt in range(KB):
                proj_T_block(wq2, KA, xTa, nt, mcc, sink_q_ca)

        # ===================== self-attention =====================
        wv = load_w("sa_wv", KB)
        proj_v(wv, ins["xb_bf"], KB)
        wq = load_w("sa_wq", KB)
        wk = load_w("sa_wk", KB)
        sink_q_sa = q_sink(qt_sa_d, bqs)
        for mcc in range(NMC):
            xT = xpose_chunk(ins["xb_bf"], KB, mcc)
            for nt in range(KB):
                proj_T_block(wq, KB, xT, nt, mcc, sink_q_sa)
                proj_T_block(wk, KB, xT, nt, mcc, k_sink)

        wo = load_w("sa_wo", KB)
        attention(qt_sa_d, sc_sa, wo, ins["xbpb"], sa_writer)

        # ===================== cross-attention =====================
        # v and k share each transposed xb1 chunk (one transpose instead of
        # two, and 2x the PE work per chunk keeps the chunk ring ahead).
        wv2 = load_w("ca_wv", KB)
        wk2 = load_w("ca_wk", KB)
        for mcc in range(NMC):
            xTb = xpose_chunk(xb1b_d, KB, mcc)
            for q2 in range(MC // P):
                mt = mcc * (MC // P) + q2
                ps = pp.tile([P, db], F32, tag="ps_s", bufs=2)
                for kt in range(KB):
                    for ecc in range(NEC):
                        nc.tensor.matmul(
                            ps[:, ecc * EC:(ecc + 1) * EC],
                            xTb[:, kt, q2 * P:(q2 + 1) * P],
                            wv2[:, kt, ecc * EC:(ecc + 1) * EC],
                            start=(kt == 0), stop=(kt == KB - 1),
                        )
                nc.vector.tensor_copy(v_sb[:, mt, :], ps[:])
            for nt in range(KB):
                proj_T_block(wk2, KB, xTb, nt, mcc, k_sink)
        wo2 = load_w("ca_wo", KB)
        attention(qt_ca_d, sc_ca, wo2, xb1_d, ca_writer)


def build_program(n=N_FULL, da=DA_FULL, db=DB_FULL, repeat=1):
    """Build the single-core Bass program; returns the Bass module.

    repeat>1 re-emits the whole block body N times (idempotent — same inputs
    and scratch): used to measure per-iteration device time above the fixed
    dispatch overhead."""
    nc = bacc.Bacc("TRN2", target_bir_lowering=False, debug=False, enable_asserts=False)
    KB = db // P
    ins = {
        "xb_bf": nc.dram_tensor("xb_bf", [n, db], BF16, kind="ExternalInput").ap(),
        "xa_bf": nc.dram_tensor("xa_bf", [n, da], BF16, kind="ExternalInput").ap(),
        "xbpb": nc.dram_tensor("xbpb", [n, db], F32, kind="ExternalInput").ap(),
        "sa_wq": nc.dram_tensor("sa_wq", [db, db], BF16, kind="ExternalInput").ap(),
        "sa_wk": nc.dram_tensor("sa_wk", [db, db], BF16, kind="ExternalInput").ap(),
        "sa_wv": nc.dram_tensor("sa_wv", [db, db], BF16, kind="ExternalInput").ap(),
        "sa_wo": nc.dram_tensor("sa_wo", [db, db], BF16, kind="ExternalInput").ap(),
        "ca_wq": nc.dram_tensor("ca_wq", [da, db], BF16, kind="ExternalInput").ap(),
        "ca_wk": nc.dram_tensor("ca_wk", [db, db], BF16, kind="ExternalInput").ap(),
        "ca_wv": nc.dram_tensor("ca_wv", [db, db], BF16, kind="ExternalInput").ap(),
        "ca_wo": nc.dram_tensor("ca_wo", [db, db], BF16, kind="ExternalInput").ap(),
        "bq_sa": nc.dram_tensor("bq_sa", [P, KB], F32, kind="ExternalInput").ap(),
        "bq_ca": nc.dram_tensor("bq_ca", [P, KB], F32, kind="ExternalInput").ap(),
    }
    outs = {"out": nc.dram_tensor("out", [n, db], F32, kind="ExternalOutput").ap()}
    with tile.TileContext(nc) as tc:
        for _ in range(repeat):
            build_block(tc, outs, ins, n, da, db)
    nc.compile()
    return nc


def prepare_maps(inputs, n=N_FULL, da=DA_FULL, db=DB_FULL):
    """Host-side prep: bf16 casts + exact bias folding.  Returns (in_maps, add_out)."""
    bf = ml_dtypes.bfloat16
    f32 = np.float32
    g = {k: np.ascontiguousarray(np.asarray(v)) for k, v in inputs.items()}
    nb = g["x_a"].shape[0]

    # exact folds (see module docstring); all biases are added in fp32
    b_eff_sa = (g["sa_bv"].astype(f32) @ g["sa_wo"].astype(f32) + g["sa_bo"].astype(f32))
    b_eff_ca = (g["ca_bv"].astype(f32) @ g["ca_wo"].astype(f32) + g["ca_bo"].astype(f32))
    xbpb = (g["x_b"].astype(f32) + b_eff_sa[None, None, :]).astype(f32)

    KB = db // P
    common = {
        "sa_wq": g["sa_wq"].astype(bf), "sa_wk": g["sa_wk"].astype(bf),
        "sa_wv": g["sa_wv"].astype(bf), "sa_wo": g["sa_wo"].astype(bf),
        "ca_wq": g["ca_wq"].astype(bf), "ca_wk": g["ca_wk"].astype(bf),
        "ca_wv": g["ca_wv"].astype(bf), "ca_wo": g["ca_wo"].astype(bf),
        "bq_sa": np.ascontiguousarray(g["sa_bq"].astype(f32).reshape(KB, P).T),
        "bq_ca": np.ascontiguousarray(g["ca_bq"].astype(f32).reshape(KB, P).T),
    }
    in_maps = []
    for b in range(nb):
        in_maps.append(dict(
            xb_bf=g["x_b"][b].astype(bf),
            xa_bf=g["x_a"][b].astype(bf),
            xbpb=np.ascontiguousarray(xbpb[b]),
            **common,
        ))
    return in_maps, b_eff_ca


_CACHE = {}


def run_on_device(inputs, trace=False, **run_kwargs):
    """Run the full problem on 8 NeuronCores.  Returns (out [B,N,DB] f32, results)."""
    if not trace:
        # NTFF tracing needs antenv.axon_hooks, absent in this container; make
        # sure an inherited BASS_TRACE=1 can't route us into that path.
        os.environ.setdefault("BASS_NEVER_TRACE", "1")
    if "nc" not in _CACHE:
        _CACHE["nc"] = build_program()
    nc = _CACHE["nc"]
    in_maps, add_out = prepare_maps(inputs)
    res = run_bass_kernel_spmd(
        nc, in_maps, core_ids=list(range(len(in_maps))), trace=trace, **run_kwargs,
    )
    out = np.stack([r["out"] for r in res.results], axis=0)
    out = (out + add_out[None, None, :]).astype(np.float32)
    return out, res


def kernel(**inputs) -> np.ndarray:
    out, _ = run_on_device(inputs)
    return out



# revision 4
# speedup vs baseline: 1.1092x; 1.1092x over previous
"""Trainium2 Bass kernel: dual-attention transformer block, fp8 DoubleRow.

Reference semantics (per batch element b):
    q/k/v = x_b @ sa_w{q,k,v} + sa_b{q,k,v}
    sa    = softmax(q k^T / sqrt(DB)) v @ sa_wo + sa_bo
    x_b1  = x_b + sa
    q     = x_a @ ca_wq + ca_bq ; k/v = x_b1 @ ca_w{k,v} + ca_b{k,v}
    out   = x_b1 + softmax(q k^T / sqrt(DA)) v @ ca_wo + ca_bo

Sharding: data-parallel over batch - 8 batch elements, one per NeuronCore.

All matmuls run in fp8e4 with MatmulPerfMode.DoubleRow (K=256 per instruction,
~2.5x bf16 throughput measured on HW).  Numerics plan:
  - weights scaled x32 on host (fp8e4 normal range starts at 2^-6); exact
    powers of two unwound via the exp scale (1/(1024*sqrt(D))), the AV
    output scale (1/64) and the softmax-reciprocal path.
  - scores are computed TRANSPOSED (s^T[j,i] = k_j . q_i) so the exp output
    lands directly in the [key, query] layout the AV matmul needs: no N x N
    transpose, no N x N normalization pass.  exp() output goes straight to
    fp8 (unnormalized weights, range ~[0.05, 20], fp8e4 max 240).
  - softmax sums (over the partition axis) via a ones-vector DoubleRow
    matmul -> [1, n] psum; a tiny DRAM round-trip re-lays [1, n] as
    [128, n/128] so the reciprocal becomes a per-partition scale applied at
    the out-projection psum (out rows = queries).
  - k-bias is softmax-invariant (dropped); v-bias/out-bias folded on host
    into the residual; q-bias added on device at the q psum->fp8 cast.
  - residual stream bf16 (x_b+b_eff as input, xb1 roundtrip); final output
    written f32.
"""

import math
import os
from contextlib import ExitStack

import numpy as np
import ml_dtypes

import concourse.bass as bass
import concourse.mybir as mybir
import concourse.tile as tile
from concourse import bacc
from concourse.bass_utils import run_bass_kernel_spmd

P = 128
F32 = mybir.dt.float32
BF16 = mybir.dt.bfloat16
F8 = mybir.dt.float8e4
AF = mybir.ActivationFunctionType
ALU = mybir.AluOpType
DR = mybir.MatmulPerfMode.DoubleRow

B_FULL, N_FULL, DA_FULL, DB_FULL = 8, 2048, 768, 1024

WS = 32.0        # host weight scale (all projection weights)
ONES_VAL = 16.0  # sums = 16*sum(e);  16 = (WS_v * WS_o) / AT_DOWN
AT_DOWN = 64.0   # AV psum -> fp8 downscale


def build_block(tc, outs, ins, n, da, db):
    """Emit the dual-attention block into TileContext `tc`.

    ins (all DRAM APs, fp8 tensors pre-scaled/pre-laid-out on host):
      xbT [P,KB,n] f8, xaT [P,KA,n] f8, xbpb [n,db] bf16,
      sa_wq/sa_wk/sa_wv/sa_wo [P,KB,db] f8, ca_wq [P,KA,db] f8,
      ca_wk/ca_wv/ca_wo [P,KB,db] f8, bq_sa/bq_ca [P,KB] f32, ones [P,2,16] f8
    outs: out [n,db] f32
    """
    nc = tc.nc
    KB, KA, NI = db // P, da // P, n // P
    SB = min(2048, n)          # attention superblock / psA span
    NSB = n // SB
    SPB = SB // P              # i-blocks per superblock
    IC = min(512, SB)          # moving chunk (DoubleRow out free dim)
    ICS = SB // IC
    ECW = min(IC, db)          # out-proj / v-proj free chunk
    NEC = db // ECW
    MC = min(256, n)           # xb1 transpose chunk
    NJP = NI // 2              # j-tile pairs (AV / sums contraction)
    assert KB % 2 == 0 and KA % 2 == 0 and NI % 2 == 0 and 2 * db <= SB

    sc_sa = 1.0 / (WS * WS * math.sqrt(float(db)))
    sc_ca = 1.0 / (WS * WS * math.sqrt(float(da)))

    ctx = ExitStack()
    with ctx:
        sp = ctx.enter_context(tc.tile_pool(name="sp", bufs=1))
        pp = ctx.enter_context(tc.tile_pool(name="pp", bufs=1, space="PSUM"))
        dp = ctx.enter_context(tc.tile_pool(name="dp", bufs=1, space="DRAM"))

        xb1b_d = dp.tile([n, db], BF16, tag="xb1b")

        # ---- persistent SBUF ----
        kT = sp.tile([P, KB, n], F8, tag="kT")      # k^T [feat, seq]
        qT = sp.tile([P, KB, n], F8, tag="qT")      # q^T (SA then CA)
        v_sb = sp.tile([P, NI, db], F8, tag="v")    # v   [seq, feat]
        xbT = sp.tile([P, KB, n], F8, tag="xbT")
        xaT = sp.tile([P, KA, n], F8, tag="xaT")
        x1T = sp.tile([P, KB, n], F8, tag="x1T")    # xb1^T fp8 for CA k/v
        bqs = sp.tile([P, KB], F32, tag="bq", bufs=2)
        bqc = sp.tile([P, KB], F32, tag="bq", bufs=2)
        ones = sp.tile([P, 2, 16], F8, tag="ones")
        nc.sync.dma_start(bqs[:], ins["bq_sa"][:])
        nc.sync.dma_start(bqc[:], ins["bq_ca"][:])
        nc.sync.dma_start(ones[:], ins["ones"][:])

        def load_w(name, ktiles):
            # two half-loads so consumers of the first k-tiles start early
            wt = sp.tile([P, KB, db], F8, tag="w", bufs=3)
            h = max(1, ktiles // 2)
            nc.sync.dma_start(wt[:, :h, :], ins[name][:, :h, :])
            if h < ktiles:
                nc.sync.dma_start(wt[:, h:ktiles, :], ins[name][:, h:ktiles, :])
            return wt

        def proj_qk(xT_sb, ktiles, targets):
            # yT[f, i] = sum_k w[k, f] xT[k, i]  (+ per-partition bias), fp8 out.
            # targets: list of (w_sb, bias_or_None, dst [P,KB,n])
            nkp = ktiles // 2
            for ih in range(n // SB):
                for fb in range(KB):
                    for (w_sb, bias, dst) in targets:
                        ps = pp.tile([P, SB], F32, tag="psA", bufs=2)
                        for kp in range(nkp):
                            for ic in range(ICS):
                                nc.tensor.matmul(
                                    ps[:, ic * IC:(ic + 1) * IC],
                                    w_sb[:, 2 * kp:2 * kp + 2, fb * P:(fb + 1) * P],
                                    xT_sb[:, 2 * kp:2 * kp + 2,
                                          ih * SB + ic * IC:ih * SB + (ic + 1) * IC],
                                    start=(kp == 0), stop=(kp == nkp - 1),
                                    perf_mode=DR,
                                )
                        d = dst[:, fb, ih * SB:(ih + 1) * SB]
                        b = 0.0 if bias is None else bias[:, fb:fb + 1]
                        nc.scalar.activation(d, ps[:], AF.Identity, bias=b)

        def proj_v(xT_sb, ktiles, w_sb, mb_range):
            # v[m, e] = sum_k xT[k, m] w[k, e], fp8 out (natural layout).
            # mb pairs share one [P, SB] psum tile (2*db <= SB).
            nkp = ktiles // 2
            for mbp in mb_range[::2]:
                ps = pp.tile([P, SB], F32, tag="psA", bufs=2)
                for off in range(2):
                    for kp in range(nkp):
                        for ec in range(NEC):
                            nc.tensor.matmul(
                                ps[:, (off * NEC + ec) * ECW:
                                   (off * NEC + ec + 1) * ECW],
                                xT_sb[:, 2 * kp:2 * kp + 2,
                                      (mbp + off) * P:(mbp + off + 1) * P],
                                w_sb[:, 2 * kp:2 * kp + 2, ec * ECW:(ec + 1) * ECW],
                                start=(kp == 0), stop=(kp == nkp - 1),
                                perf_mode=DR,
                            )
                nc.vector.tensor_copy(v_sb[:, mbp:mbp + 2, :], ps[:, :2 * db])

        def attention(scale, wo_sb, resid_dram, writer, hooks):
            # hooks: {emission_point_name: fn()} to interleave other phases
            nkp = KB // 2
            ndp = KB // 2

            def scores(sb, wt_t):
                for jb in range(NI):
                    ps = pp.tile([P, SB], F32, tag="psA", bufs=2)
                    for kp in range(nkp):
                        for ic in range(ICS):
                            nc.tensor.matmul(
                                ps[:, ic * IC:(ic + 1) * IC],
                                kT[:, 2 * kp:2 * kp + 2, jb * P:(jb + 1) * P],
                                qT[:, 2 * kp:2 * kp + 2,
                                   sb * SB + ic * IC:sb * SB + (ic + 1) * IC],
                                start=(kp == 0), stop=(kp == nkp - 1),
                                perf_mode=DR,
                            )
                    nc.scalar.activation(wt_t[:, jb, :], ps[:], AF.Exp,
                                         bias=0.0, scale=scale)

            def sums(sb, wt_t, rr_t):
                sums_t = dp.tile([SB], F32, tag="sums", bufs=4)
                pss = pp.tile([P, SB], F32, tag="psA", bufs=2, name="ps_sum")
                for ic in range(ICS):
                    for jp in range(NJP):
                        nc.tensor.matmul(
                            pss[0:1, ic * IC:(ic + 1) * IC],
                            ones[:, :, 0:1],
                            wt_t[:, 2 * jp:2 * jp + 2, ic * IC:(ic + 1) * IC],
                            start=(jp == 0), stop=(jp == NJP - 1),
                            perf_mode=DR,
                        )
                ssb = sp.tile([1, SB], F32, tag="ss", bufs=1)
                nc.vector.tensor_copy(ssb[:], pss[0:1, :])
                nc.sync.dma_start(sums_t[:], ssb[:])
                rrb = sp.tile([P, SPB], F32, tag="rrb", bufs=2)
                nc.sync.dma_start(rrb[:], sums_t.rearrange("(t p) -> p t", p=P))
                nc.vector.reciprocal(rr_t[:], rrb[:])

            def av(sb, wt_t, at_t):
                for dt in range(KB):
                    pss = pp.tile([P, SB], F32, tag="psA", bufs=2, name="ps_av")
                    for jp in range(NJP):
                        for ic in range(ICS):
                            nc.tensor.matmul(
                                pss[:, ic * IC:(ic + 1) * IC],
                                v_sb[:, 2 * jp:2 * jp + 2, dt * P:(dt + 1) * P],
                                wt_t[:, 2 * jp:2 * jp + 2, ic * IC:(ic + 1) * IC],
                                start=(jp == 0), stop=(jp == NJP - 1),
                                perf_mode=DR,
                            )
                    nc.vector.tensor_scalar_mul(at_t[:, dt, :], pss[:],
                                                1.0 / AT_DOWN)

            def outproj(sb, at_t, rr_t, rx_list):
                for q3p in range(0, SPB, 2):
                    pso = pp.tile([P, SB], F32, tag="psA", bufs=2, name="ps_op")
                    for off in range(2):
                        q3 = q3p + off
                        for dp_ in range(ndp):
                            for ec in range(NEC):
                                sl = (off * NEC + ec) * ECW
                                nc.tensor.matmul(
                                    pso[:, sl:sl + ECW],
                                    at_t[:, 2 * dp_:2 * dp_ + 2,
                                         q3 * P:(q3 + 1) * P],
                                    wo_sb[:, 2 * dp_:2 * dp_ + 2,
                                          ec * ECW:(ec + 1) * ECW],
                                    start=(dp_ == 0), stop=(dp_ == ndp - 1),
                                    perf_mode=DR,
                                )
                    for off in range(2):
                        q3 = q3p + off
                        ib = sb * SPB + q3
                        ro = sp.tile([P, db], writer.dtype, tag=writer.tag,
                                     bufs=2, name="ro")
                        nc.scalar.activation(
                            ro[:], pso[:, off * db:(off + 1) * db], AF.Identity,
                            bias=0.0, scale=rr_t[:, q3:q3 + 1])
                        nc.vector.tensor_tensor(ro[:], ro[:], rx_list[q3][:],
                                                ALU.add)
                        writer(ib, ro)
                    if "after_pair" in hooks:
                        hooks["after_pair"](sb * SPB + q3p)

            def rx_load(sb):
                lst = []
                for q3 in range(SPB):
                    ib = sb * SPB + q3
                    rx = sp.tile([P, db], BF16, tag="rx", bufs=min(SPB, 6))
                    nc.sync.dma_start(rx[:], resid_dram[ib * P:(ib + 1) * P, :])
                    lst.append(rx)
                return lst

            def tail(sb):
                # sums -> (hook) -> rx prefetch -> AV -> out-proj for sb
                sums(sb, wt[sb], rr[sb])
                if sb == 0 and "after_sums0" in hooks:
                    hooks["after_sums0"]()
                rx = rx_load(sb)
                at_t = sp.tile([P, KB, SB], F8, tag="at", bufs=min(NSB, 2), name="at_t")
                av(sb, wt[sb], at_t)
                outproj(sb, at_t, rr[sb], rx)
                if "after_op" in hooks:
                    hooks["after_op"](sb)

            wt, rr = {}, {}
            for sb in range(NSB):
                wt[sb] = sp.tile([P, NI, SB], F8, tag="wt", bufs=min(NSB, 2), name="wt_t")
                rr[sb] = sp.tile([P, SPB], F32, tag="rr", bufs=2, name="rr_t")
                scores(sb, wt[sb])
                if sb >= 1:
                    tail(sb - 1)
            tail(NSB - 1)

        def sa_writer(ib, ro):
            nc.sync.dma_start(xb1b_d[ib * P:(ib + 1) * P, :], ro[:])
        sa_writer.dtype, sa_writer.tag = BF16, "roA"

        def ca_writer(ib, ro):
            nc.sync.dma_start(outs["out"][ib * P:(ib + 1) * P, :], ro[:])
        ca_writer.dtype, ca_writer.tag = F32, "roB"

        # ===================== self-attention =====================
        wq = load_w("sa_wq", KB)
        wk = load_w("sa_wk", KB)
        nc.sync.dma_start(xbT[:], ins["xbT"][:])
        proj_qk(xbT, KB, [(wq, bqs, qT), (wk, None, kT)])
        wv = load_w("sa_wv", KB)
        proj_v(xbT, KB, wv, range(NI))
        wo = load_w("sa_wo", KB)

        # CA q (from x_a) is emitted inside the SA attention via hook: it
        # fills the PE while SA's softmax tail / rr round-trips complete.
        nc.sync.dma_start(xaT[:, :KA, :], ins["xaT"][:])
        wq2 = load_w("ca_wq", KA)

        def emit_ca_q():
            proj_qk(xaT, KA, [(wq2, bqc, qT)])

        def emit_xpose(ibp):
            # transpose the xb1 rows finished by out-proj pair starting at ibp
            r0 = ibp * P
            for mcc in range(r0 // MC, (r0 + 2 * P) // MC):
                xTb = sp.tile([P, KB, MC], BF16, tag="xtb", bufs=2)
                nc.sync.dma_start_transpose(
                    xTb[:], xb1b_d[mcc * MC:(mcc + 1) * MC, :])
                nc.vector.tensor_copy(x1T[:, :, mcc * MC:(mcc + 1) * MC], xTb[:])

        attention(sc_sa, wo, ins["xbpb"], sa_writer,
                  {"after_sums0": emit_ca_q, "after_pair": emit_xpose})

        # ===================== cross-attention =====================
        wk2 = load_w("ca_wk", KB)
        proj_qk(x1T, KB, [(wk2, None, kT)])
        wv2 = load_w("ca_wv", KB)
        proj_v(x1T, KB, wv2, range(NI))
        wo2 = load_w("ca_wo", KB)
        attention(sc_ca, wo2, xb1b_d, ca_writer, {})


def build_program(n=N_FULL, da=DA_FULL, db=DB_FULL, repeat=1):
    nc = bacc.Bacc("TRN2", target_bir_lowering=False, debug=False, enable_asserts=False)
    KB, KA = db // P, da // P
    ins = {
        "xbT": nc.dram_tensor("xbT", [P, KB, n], F8, kind="ExternalInput").ap(),
        "xaT": nc.dram_tensor("xaT", [P, KA, n], F8, kind="ExternalInput").ap(),
        "xbpb": nc.dram_tensor("xbpb", [n, db], BF16, kind="ExternalInput").ap(),
        "bq_sa": nc.dram_tensor("bq_sa", [P, KB], F32, kind="ExternalInput").ap(),
        "bq_ca": nc.dram_tensor("bq_ca", [P, KB], F32, kind="ExternalInput").ap(),
        "ones": nc.dram_tensor("ones", [P, 2, 16], F8, kind="ExternalInput").ap(),
    }
    for nm in ("sa_wq", "sa_wk", "sa_wv", "sa_wo", "ca_wk", "ca_wv", "ca_wo"):
        ins[nm] = nc.dram_tensor(nm, [P, KB, db], F8, kind="ExternalInput").ap()
    ins["ca_wq"] = nc.dram_tensor("ca_wq", [P, KA, db], F8, kind="ExternalInput").ap()
    outs = {"out": nc.dram_tensor("out", [n, db], F32, kind="ExternalOutput").ap()}
    with tile.TileContext(nc) as tc:
        for _ in range(repeat):
            build_block(tc, outs, ins, n, da, db)
    nc.compile()
    return nc


def _f8(a):
    return np.clip(a, -240.0, 240.0).astype(ml_dtypes.float8_e4m3)


def _wlay(w, P_=P):
    # [din, dout] -> [P, din/P, dout], k = kt*P + p
    din, dout = w.shape
    return np.ascontiguousarray(w.reshape(din // P_, P_, dout).transpose(1, 0, 2))


def prepare_maps(inputs, n=N_FULL, da=DA_FULL, db=DB_FULL):
    """Host-side prep: fp8 scaling/layout + exact bias folding."""
    f32 = np.float32
    bf = ml_dtypes.bfloat16
    g = {k: np.ascontiguousarray(np.asarray(v)) for k, v in inputs.items()}
    nb = g["x_a"].shape[0]
    KB = db // P

    b_eff_sa = (g["sa_bv"].astype(f32) @ g["sa_wo"].astype(f32) + g["sa_bo"].astype(f32))
    b_eff_ca = (g["ca_bv"].astype(f32) @ g["ca_wo"].astype(f32) + g["ca_bo"].astype(f32))
    xbpb = (g["x_b"].astype(f32) + b_eff_sa[None, None, :]).astype(bf)

    ones = np.full((P, 2, 16), ONES_VAL, ml_dtypes.float8_e4m3)
    common = {"ones": ones}
    for nm in ("sa_wq", "sa_wk", "sa_wv", "sa_wo", "ca_wq", "ca_wk", "ca_wv", "ca_wo"):
        common[nm] = _f8(_wlay(g[nm].astype(f32) * WS))
    common["bq_sa"] = np.ascontiguousarray(
        (g["sa_bq"].astype(f32) * WS).reshape(KB, P).T)
    common["bq_ca"] = np.ascontiguousarray(
        (g["ca_bq"].astype(f32) * WS).reshape(KB, P).T)

    in_maps = []
    for b in range(nb):
        xbT = _f8(np.ascontiguousarray(
            g["x_b"][b].T.astype(f32).reshape(db // P, P, n).transpose(1, 0, 2)))
        xaT = _f8(np.ascontiguousarray(
            g["x_a"][b].T.astype(f32).reshape(da // P, P, n).transpose(1, 0, 2)))
        in_maps.append(dict(
            xbT=xbT, xaT=xaT, xbpb=np.ascontiguousarray(xbpb[b]), **common,
        ))
    return in_maps, b_eff_ca


_CACHE = {}


def run_on_device(inputs, trace=False, **run_kwargs):
    if not trace:
        os.environ.setdefault("BASS_NEVER_TRACE", "1")
    if "nc" not in _CACHE:
        _CACHE["nc"] = build_program()
    nc = _CACHE["nc"]
    in_maps, add_out = prepare_maps(inputs)
    res = run_bass_kernel_spmd(
        nc, in_maps, core_ids=list(range(len(in_maps))), trace=trace, **run_kwargs,
    )
    out = np.stack([r["out"] for r in res.results], axis=0)
    out = (out + add_out[None, None, :]).astype(np.float32)
    return out, res


def kernel(**inputs) -> np.ndarray:
    out, _ = run_on_device(inputs)
    return out
